# revision 1
# baseline (speedup 1.0000x reference)
"""Median graph convolution on 8 Trainium2 NeuronCores.

out[n, c] = median over valid neighbors j of (x @ kernel)[neighbors[n, j], c]
(lower median, rank (deg-1)//2 of the first deg neighbor slots).

Strategy (data-parallel over nodes, 6272 nodes/core):
  - host ships x^T pre-cast to fp16 (layout marshaling; all FLOPs on device)
  - each core matmuls its node shard on the PE -> h shard (fp16)
  - AllGather h shards into a per-core HBM table with +-inf sentinel rows
  - dma_gather pulls 32 neighbor rows per node into SBUF in
    [node->partition, (k, c)] layout.  Indices are int16, so the 50k-row
    table is addressed through two views (rows 0..32767 and 32768..end);
    each slot gathers once from each view, with the "wrong" view pointed at
    a -inf sentinel row, and one elementwise max merges them.
  - degree padding also rides the index stream: p_lo -inf and p_hi +inf
    sentinels per node pin the target at fixed rank 15 of 32.
  - a batched bitonic network (two sorted 16s + the classic two-way
    merge rank formula) computes rank 15 on the Vector engine in fp16.
"""

import sys

sys.path.insert(0, "/opt/trn_rl_repo")

import numpy as np

N, K, IN_C, OUT_C = 50000, 32, 256, 128
NCORES = 8
NTILES = 49                      # 128-node tiles per core
SHARD = NTILES * 128             # 6272
NPAD = SHARD * NCORES            # 50176
# gather table layout: row 0 = -inf, rows 1..NPAD = h, then +inf, -inf
TROWS = NPAD + 3
ROW_PINF = NPAD + 1              # 50177
ROW_MINF2 = NPAD + 2             # 50178
BSPLIT = 32768                   # B view = table[BSPLIT:]
B_PINF = ROW_PINF - BSPLIT       # +inf row in B view
B_MINF = ROW_MINF2 - BSPLIT      # -inf row in B view
SLOTS = 4096                     # 32 slots * 128 nodes per tile
MM_CHUNK = 448                   # matmul free-dim chunk (14 * 448 = 6272)

_CACHE = {}


def _emit_program():
    import concourse.tile as tile
    import concourse.mybir as mybir
    from concourse import bacc
    from concourse.bass import AP
    from concourse.library_config import mlp

    fp16 = mybir.dt.float16
    fp32 = mybir.dt.float32
    i16 = mybir.dt.int16
    Alu = mybir.AluOpType

    nc = bacc.Bacc("TRN2", target_bir_lowering=False, num_swdge_queues=4, dynamic_dma_scratch_size=32768)

    xT = nc.dram_tensor("xT", [IN_C, SHARD], fp16, kind="ExternalInput")
    w = nc.dram_tensor("w", [IN_C, OUT_C], fp16, kind="ExternalInput")
    idxA = nc.dram_tensor("idxA", [NTILES, 128, SLOTS // 16], i16, kind="ExternalInput")
    idxB = nc.dram_tensor("idxB", [NTILES, 128, SLOTS // 16], i16, kind="ExternalInput")
    infs = nc.dram_tensor("infs", [2, OUT_C], fp16, kind="ExternalInput")  # [+inf, -inf]
    out = nc.dram_tensor("out", [SHARD, OUT_C], fp32, kind="ExternalOutput")
    table = nc.dram_tensor("table", [TROWS, OUT_C], fp16)
    hshard = nc.dram_tensor("hshard", [SHARD, OUT_C], fp16)

    S = OUT_C  # slot stride in elements inside a [128, 32*128] value tile

    def slot_ap(t, slot0, dims):
        """AP over value tile t: partition dim + given (slot_step, count) dims + c."""
        base = t[:]
        free = [[st * S, ct] for (st, ct) in dims if ct != 1]
        return AP(base.tensor, base.offset + slot0 * S, [base.ap[0]] + free + [[1, OUT_C]])

    with tile.TileContext(nc) as tc:
        nc.gpsimd.load_library(mlp)
        with (
            tc.tile_pool(name="const", bufs=1) as cpool,
            tc.tile_pool(name="psum", bufs=2, space="PSUM") as psum_pool,
            tc.tile_pool(name="gbuf", bufs=3) as gpool,
            tc.tile_pool(name="work", bufs=2) as wpool,
        ):
            # ---- phase 1: h rows = x @ w directly (x chunk is the PE
            # stationary operand, so PSUM comes out in [node, c] layout) ----
            with tc.tile_pool(name="stage", bufs=1) as spool:
                lw0 = spool.tile([128, OUT_C], fp16)
                lw1 = spool.tile([128, OUT_C], fp16)
                nc.sync.dma_start(lw0[:], w[0:128, :])
                nc.sync.dma_start(lw1[:], w[128:256, :])
                xt0 = spool.tile([128, SHARD], fp16)
                xt1 = spool.tile([128, SHARD], fp16)
                nc.sync.dma_start(xt0[:], xT[0:128, :])
                nc.sync.dma_start(xt1[:], xT[128:256, :])
                hrows = spool.tile([128, NTILES, OUT_C], fp16)
                for j in range(NTILES):
                    ns = slice(j * 128, (j + 1) * 128)
                    ps = psum_pool.tile([128, OUT_C], fp32)
                    nc.tensor.matmul(ps[:], lhsT=xt0[:, ns], rhs=lw0[:], start=True, stop=False)
                    nc.tensor.matmul(ps[:], lhsT=xt1[:, ns], rhs=lw1[:], start=False, stop=True)
                    nc.vector.tensor_copy(hrows[:, j, :], ps[:])
                nc.sync.dma_start(
                    hshard[:].rearrange("(j n) c -> n j c", n=128), hrows[:]
                )

            # ---- phase 3: AllGather shards into the table; write sentinels ----
            nc.gpsimd.collective_compute(
                "AllGather",
                mybir.AluOpType.bypass,
                replica_groups=[list(range(NCORES))],
                ins=[hshard[:]],
                outs=[table[1:NPAD + 1, :]],
            )
            inft = cpool.tile([2, OUT_C], fp16)
            nc.sync.dma_start(inft[:], infs[:])
            nc.sync.dma_start(table[0:1, :], inft[1:2, :])
            nc.sync.dma_start(table[ROW_PINF:ROW_PINF + 1, :], inft[0:1, :])
            nc.sync.dma_start(table[ROW_MINF2:ROW_MINF2 + 1, :], inft[1:2, :])

            # ---- phase 4: gather + median per 128-node tile ----
            # bitonic stages sorting slots 0..15 and 16..31 ascending
            stages = []
            for k in (2, 4, 8, 16):
                j = k // 2
                while j >= 1:
                    stages.append((k, j))
                    j //= 2

            iaall = cpool.tile([128, NTILES * (SLOTS // 16)], i16)
            iball = cpool.tile([128, NTILES * (SLOTS // 16)], i16)
            nc.sync.dma_start(
                iaall[:].rearrange("p (t s) -> p t s", t=NTILES),
                idxA[:].rearrange("t p s -> p t s"))
            nc.sync.dma_start(
                iball[:].rearrange("p (t s) -> p t s", t=NTILES),
                idxB[:].rearrange("t p s -> p t s"))
            SW = SLOTS // 16
            for t in range(NTILES):
                bufA = gpool.tile([128, K, OUT_C], fp16, tag="bufA")
                bufB = gpool.tile([128, K, OUT_C], fp16, tag="bufB")
                # dma_gather crashes above ~1k descriptors/call; chunk it
                GC = 1024
                for ci, off in enumerate(range(0, SLOTS, GC)):
                    nc.gpsimd.dma_gather(
                        bufA[:, off // 128:(off + GC) // 128, :],
                        table[:], iaall[:, t * SW + off // 16:t * SW + (off + GC) // 16],
                        GC, GC, OUT_C, queue_num=(2 * ci) % 4, single_packet=False)
                    nc.gpsimd.dma_gather(
                        bufB[:, off // 128:(off + GC) // 128, :],
                        table[BSPLIT:, :], iball[:, t * SW + off // 16:t * SW + (off + GC) // 16],
                        GC, GC, OUT_C, queue_num=(2 * ci + 1) % 4, single_packet=False)

                v0 = wpool.tile([128, K, OUT_C], fp16, tag="v0")
                v1 = wpool.tile([128, K, OUT_C], fp16, tag="v1")
                nc.vector.tensor_tensor(
                    out=v0[:].rearrange("p k c -> p (k c)"),
                    in0=bufA[:].rearrange("p k c -> p (k c)"),
                    in1=bufB[:].rearrange("p k c -> p (k c)"),
                    op=Alu.max,
                )

                src, dst = v0, v1
                for (k, j) in stages:
                    if k == 16:
                        # all comparators ascending: lows = {i: (i & j) == 0}
                        lo = [(2 * j, 32 // (2 * j)), (1, j)]
                        nc.vector.tensor_tensor(
                            out=slot_ap(dst, 0, lo),
                            in0=slot_ap(src, 0, lo),
                            in1=slot_ap(src, j, lo),
                            op=Alu.min,
                        )
                        nc.vector.tensor_tensor(
                            out=slot_ap(dst, j, lo),
                            in0=slot_ap(src, 0, lo),
                            in1=slot_ap(src, j, lo),
                            op=Alu.max,
                        )
                    else:
                        dims = [(2 * k, 32 // (2 * k)), (2 * j, k // (2 * j)), (1, j)]
                        for desc in (0, 1):
                            base = k if desc else 0
                            lo_out, hi_out = (j, 0) if desc else (0, j)
                            nc.vector.tensor_tensor(
                                out=slot_ap(dst, base + lo_out, dims),
                                in0=slot_ap(src, base, dims),
                                in1=slot_ap(src, base + j, dims),
                                op=Alu.min,
                            )
                            nc.vector.tensor_tensor(
                                out=slot_ap(dst, base + hi_out, dims),
                                in0=slot_ap(src, base, dims),
                                in1=slot_ap(src, base + j, dims),
                                op=Alu.max,
                            )
                    src, dst = dst, src
                # sorted halves now in `src`: A = slots 0..15 asc, B = 16..31 asc
                # rank15(A u B) = min(A15, B15, min_t max(A[t], B[14-t]))
                m = wpool.tile([128, 17, OUT_C], fp16, tag="m")
                nc.vector.tensor_tensor(
                    out=slot_ap(m, 0, [(1, 15)]),
                    in0=slot_ap(src, 0, [(1, 15)]),
                    in1=AP(
                        src[:].tensor,
                        src[:].offset + 30 * S,
                        [src[:].ap[0], [-S, 15], [1, OUT_C]],
                    ),
                    op=Alu.max,
                )
                nc.vector.tensor_copy(slot_ap(m, 15, [(1, 2)]), slot_ap(src, 15, [(16, 2)]))
                nc.vector.tensor_tensor(
                    out=slot_ap(m, 0, [(1, 8)]),
                    in0=slot_ap(m, 0, [(1, 8)]),
                    in1=slot_ap(m, 8, [(1, 8)]),
                    op=Alu.min,
                )
                nc.vector.tensor_tensor(
                    out=slot_ap(m, 0, [(1, 4)]),
                    in0=slot_ap(m, 0, [(1, 4)]),
                    in1=slot_ap(m, 4, [(1, 4)]),
                    op=Alu.min,
                )
                nc.vector.tensor_tensor(
                    out=slot_ap(m, 0, [(1, 2)]),
                    in0=slot_ap(m, 0, [(1, 2)]),
                    in1=slot_ap(m, 2, [(1, 2)]),
                    op=Alu.min,
                )
                nc.vector.tensor_tensor(
                    out=slot_ap(m, 0, [(1, 1)]),
                    in0=slot_ap(m, 0, [(1, 1)]),
                    in1=slot_ap(m, 1, [(1, 1)]),
                    op=Alu.min,
                )
                o = wpool.tile([128, OUT_C], fp32, tag="o")
                nc.vector.tensor_tensor(
                    out=o[:],
                    in0=slot_ap(m, 0, [(1, 1)]),
                    in1=slot_ap(m, 16, [(1, 1)]),
                    op=Alu.min,
                )
                nc.sync.dma_start(out[t * 128:(t + 1) * 128, :], o[:])

    nc.compile()
    return nc


def _build_indices(neighbors, degrees):
    """Per-core wrapped int16 index arrays for the A and B gathers."""
    deg = degrees.astype(np.int64)
    p_lo = 15 - (deg - 1) // 2                      # [N]
    karr = np.arange(K, dtype=np.int64)[None, :]    # [1, K]
    valid = karr < deg[:, None]                     # [N, K]
    nbr = neighbors.astype(np.int64)
    row = nbr + 1

    a_val = np.where(valid & (row <= 32767), row, 0)
    b_real = np.where(valid & (row >= BSPLIT), row - BSPLIT, B_MINF)
    b_pad = np.where(karr < (deg + p_lo)[:, None], B_MINF, B_PINF)
    b_val = np.where(valid, b_real, b_pad)

    a_full = np.zeros((NPAD, K), np.int16)
    b_full = np.full((NPAD, K), B_MINF, np.int16)
    a_full[:N] = a_val.astype(np.int16)
    b_full[:N] = b_val.astype(np.int16)

    def wrap(arr):
        # [NPAD, K] -> per-core [NTILES, 128 nodes, K] -> flat pos i = k*128+n
        a = arr.reshape(NCORES, NTILES, 128, K).transpose(0, 1, 3, 2)  # [.., k, n]
        a = a.reshape(NCORES, NTILES, SLOTS)
        # wrapped: pos i -> [i % 16, i // 16], replicated on 8 partition groups
        a = a.reshape(NCORES, NTILES, SLOTS // 16, 16).transpose(0, 1, 3, 2)
        a = np.broadcast_to(a[:, :, None], (NCORES, NTILES, 8, 16, SLOTS // 16))
        return np.ascontiguousarray(a).reshape(NCORES, NTILES, 128, SLOTS // 16)

    return wrap(a_full), wrap(b_full)


def kernel(x, kernel, neighbors, degrees):
    from concourse.bass_utils import run_bass_kernel_spmd

    if "nc" not in _CACHE:
        _CACHE["nc"] = _emit_program()
    nc = _CACHE["nc"]

    xT = np.zeros((IN_C, NPAD), np.float16)
    xT[:, :N] = np.asarray(x, np.float32).T.astype(np.float16)
    wf = np.asarray(kernel, np.float32).astype(np.float16)
    infs = np.stack([np.full(OUT_C, np.inf, np.float16),
                     np.full(OUT_C, -np.inf, np.float16)])
    ia, ib = _build_indices(np.asarray(neighbors), np.asarray(degrees))

    in_maps = [
        {
            "xT": np.ascontiguousarray(xT[:, c * SHARD:(c + 1) * SHARD]),
            "w": wf,
            "idxA": ia[c],
            "idxB": ib[c],
            "infs": infs,
        }
        for c in range(NCORES)
    ]
    res = run_bass_kernel_spmd(nc, in_maps, list(range(NCORES)))
    full = np.concatenate([res.results[c]["out"] for c in range(NCORES)], axis=0)
    return np.ascontiguousarray(full[:N]).astype(np.float32)



# revision 2
# speedup vs baseline: 5.1741x; 5.1741x over previous
"""Median graph convolution on 8 Trainium2 NeuronCores.

out[n, c] = median over valid neighbors j of (x @ kernel)[neighbors[n, j], c]
(lower median, rank (deg-1)//2 of the first deg neighbor slots).

Strategy (data-parallel over nodes, 6272 nodes/core):
  - host sorts nodes by degree (descending), striped across the 8 cores so
    every core sees the same degree profile and one compiled program fits all
  - each core matmuls its node shard on the PE -> h shard (fp16),
    AllGather into a per-core HBM table with trailing +inf sentinel rows
  - the table is indexed as 512-byte PAIR rows (two h rows per descriptor),
    so the 50176-row table needs only 25089 int16-indexable pair rows;
    each real neighbor costs exactly ONE gather descriptor
  - only the first maxdeg(tile) slots are gathered per 128-node tile
    (pads ride as +inf sentinel descriptors / vector memset)
  - a copy + copy_predicated (int16 parity mask, stride-0 broadcast over
    channels) selects the wanted half of each gathered pair
  - a degree-adaptive bitonic network sorts the two H-halves of the slot
    array and a rank-r two-way merge formula extracts the lower median
"""

import sys

sys.path.insert(0, "/opt/trn_rl_repo")

import numpy as np

N, K, IN_C, OUT_C = 50000, 32, 256, 128
NCORES = 8
NTILES = 49                      # 128-node tiles per core
SHARD = NTILES * 128             # 6272
NPAD = SHARD * NCORES            # 50176
TROWS = NPAD + 4                 # +inf sentinel rows at the end
SENT_PAIR = NPAD // 2            # pair index of the +inf sentinel row pair
NPAIRS = SENT_PAIR + 1           # pair rows addressable by the gather
GCHUNK = 8                       # slots per dma_gather call (8*128 = 1024 idx)
MAXSLOTS = 32

_CACHE = {}


def _next_pow2(x):
    p = 1
    while p < x:
        p *= 2
    return p


def _make_schedule(deg_sorted):
    """Per-tile (maxd, H, r_list) from the global descending degree profile."""
    sched = []
    for t in range(NTILES):
        degs = deg_sorted[t * 128 * NCORES:(t + 1) * 128 * NCORES]
        maxd = int(degs[0])
        H = max(1, _next_pow2(maxd) // 2)
        rs = sorted({int((d - 1) // 2) for d in degs}, reverse=True)
        sched.append((maxd, H, tuple(rs)))
    return tuple(sched)


def _emit_program(sched):
    import concourse.tile as tile
    import concourse.mybir as mybir
    from concourse import bacc
    from concourse.bass import AP
    from concourse.library_config import mlp

    fp16 = mybir.dt.float16
    fp32 = mybir.dt.float32
    i16 = mybir.dt.int16
    Alu = mybir.AluOpType

    tot_idx_cols = sum(maxd * 8 for (maxd, _, _) in sched)
    tot_par_cols = sum(maxd for (maxd, _, _) in sched)
    tot_pick = sum(len(rs) - 1 for (_, _, rs) in sched)

    nc = bacc.Bacc("TRN2", target_bir_lowering=False, num_swdge_queues=4,
                   dynamic_dma_scratch_size=32768)

    xT = nc.dram_tensor("xT", [IN_C, SHARD], fp16, kind="ExternalInput")
    w = nc.dram_tensor("w", [IN_C, OUT_C], fp16, kind="ExternalInput")
    idx_d = nc.dram_tensor("idx", [128, tot_idx_cols], i16, kind="ExternalInput")
    par_d = nc.dram_tensor("par", [128, tot_par_cols], i16, kind="ExternalInput")
    pick_d = nc.dram_tensor("pick", [128, max(1, tot_pick)], i16, kind="ExternalInput")
    infs = nc.dram_tensor("infs", [4, OUT_C], fp16, kind="ExternalInput")  # +inf rows
    out = nc.dram_tensor("out", [SHARD, OUT_C], fp32, kind="ExternalOutput")
    table = nc.dram_tensor("table", [TROWS, OUT_C], fp16)
    hshard = nc.dram_tensor("hshard", [SHARD, OUT_C], fp16)

    # gather source: the table viewed as 512B pair rows [NPAIRS, 256]
    pair_ap = AP(table[:].tensor, 0, [[2 * OUT_C, NPAIRS], [1, 2 * OUT_C]])

    S = OUT_C  # slot stride (elements) in the selected-value tile v

    def slot_ap(t, slot0, dims):
        """AP over value tile t: partition dim + (slot_step, count) dims + c."""
        base = t[:]
        free = [[st * S, ct] for (st, ct) in dims if ct != 1]
        return AP(base.tensor, base.offset + slot0 * S, [base.ap[0]] + free + [[1, OUT_C]])

    def sort_stages(H):
        ks = []
        k = 2
        while k <= H:
            j = k // 2
            while j >= 1:
                ks.append((k, j))
                j //= 2
            k *= 2
        return ks

    with tile.TileContext(nc) as tc:
        nc.gpsimd.load_library(mlp)
        with (
            tc.tile_pool(name="const", bufs=1) as cpool,
            tc.tile_pool(name="psum", bufs=2, space="PSUM") as psum_pool,
            tc.tile_pool(name="gbuf", bufs=4) as gpool,
            tc.tile_pool(name="work", bufs=2) as wpool,
            tc.tile_pool(name="mout", bufs=2) as mpool,
        ):
            # ---- phase 1: h rows = x @ w (x chunk stationary -> [node, c]) ----
            with tc.tile_pool(name="stage", bufs=1) as spool:
                lw0 = spool.tile([128, OUT_C], fp16)
                lw1 = spool.tile([128, OUT_C], fp16)
                nc.sync.dma_start(lw0[:], w[0:128, :])
                nc.sync.dma_start(lw1[:], w[128:256, :])
                xt0 = spool.tile([128, SHARD], fp16)
                xt1 = spool.tile([128, SHARD], fp16)
                nc.sync.dma_start(xt0[:], xT[0:128, :])
                nc.sync.dma_start(xt1[:], xT[128:256, :])
                hrows = spool.tile([128, NTILES, OUT_C], fp16)
                for j in range(NTILES):
                    ns = slice(j * 128, (j + 1) * 128)
                    ps = psum_pool.tile([128, OUT_C], fp32)
                    nc.tensor.matmul(ps[:], lhsT=xt0[:, ns], rhs=lw0[:], start=True, stop=False)
                    nc.tensor.matmul(ps[:], lhsT=xt1[:, ns], rhs=lw1[:], start=False, stop=True)
                    nc.vector.tensor_copy(hrows[:, j, :], ps[:])
                nc.sync.dma_start(
                    hshard[:].rearrange("(j n) c -> n j c", n=128), hrows[:]
                )

            # ---- phase 2: AllGather shards into the table; +inf sentinels ----
            nc.gpsimd.collective_compute(
                "AllGather",
                mybir.AluOpType.bypass,
                replica_groups=[list(range(NCORES))],
                ins=[hshard[:]],
                outs=[table[0:NPAD, :]],
            )
            inft = cpool.tile([4, OUT_C], fp16)
            nc.sync.dma_start(inft[:], infs[:])
            nc.sync.dma_start(table[NPAD:NPAD + 4, :], inft[:])

            # ---- load index/mask streams ----
            idx_sb = cpool.tile([128, tot_idx_cols], i16)
            par_sb = cpool.tile([128, tot_par_cols], i16)
            pick_sb = cpool.tile([128, max(1, tot_pick)], i16)
            nc.sync.dma_start(idx_sb[:], idx_d[:])
            nc.sync.dma_start(par_sb[:], par_d[:])
            nc.sync.dma_start(pick_sb[:], pick_d[:])

            # ---- phase 3: gather + select + sort + median per tile ----
            icol = 0      # running idx column offset
            pcol = 0      # running parity column offset
            kcol = 0      # running pick-mask column offset
            qn = 0        # dma queue rotation
            for t, (maxd, H, rs) in enumerate(sched):
                P2 = 2 * H
                buf = gpool.tile([128, MAXSLOTS, 2 * OUT_C], fp16, tag="pair")
                for s0 in range(0, maxd, GCHUNK):
                    s1 = min(s0 + GCHUNK, maxd)
                    G = (s1 - s0) * 128
                    nc.gpsimd.dma_gather(
                        buf[:, s0:s1, :],
                        pair_ap,
                        idx_sb[:, icol + s0 * 8: icol + s1 * 8],
                        G, G, 2 * OUT_C,
                        queue_num=qn, single_packet=False)
                    qn = (qn + 1) % 4
                icol += maxd * 8

                v0 = wpool.tile([128, MAXSLOTS, OUT_C], fp16, tag="v0")
                v1 = wpool.tile([128, MAXSLOTS, OUT_C], fp16, tag="v1")
                bb = buf[:]
                a_ap = AP(bb.tensor, bb.offset, [bb.ap[0], [2 * OUT_C, maxd], [1, OUT_C]])
                b_ap = AP(bb.tensor, bb.offset + OUT_C, [bb.ap[0], [2 * OUT_C, maxd], [1, OUT_C]])
                pp = par_sb[:]
                m_ap = AP(pp.tensor, pp.offset + pcol, [pp.ap[0], [1, maxd], [0, OUT_C]])
                pcol += maxd
                nc.vector.tensor_copy(slot_ap(v0, 0, [(1, maxd)]), a_ap)
                nc.vector.copy_predicated(slot_ap(v0, 0, [(1, maxd)]), m_ap, b_ap)
                if maxd < P2:
                    nc.vector.memset(slot_ap(v0, maxd, [(1, P2 - maxd)]), float("inf"))

                src, dst = v0, v1
                for (k, j) in sort_stages(H):
                    if k == H:
                        lo = [(2 * j, P2 // (2 * j)), (1, j)]
                        nc.vector.tensor_tensor(
                            out=slot_ap(dst, 0, lo),
                            in0=slot_ap(src, 0, lo),
                            in1=slot_ap(src, j, lo),
                            op=Alu.min,
                        )
                        nc.vector.tensor_tensor(
                            out=slot_ap(dst, j, lo),
                            in0=slot_ap(src, 0, lo),
                            in1=slot_ap(src, j, lo),
                            op=Alu.max,
                        )
                    else:
                        dims = [(2 * k, P2 // (2 * k)), (2 * j, k // (2 * j)), (1, j)]
                        for desc in (0, 1):
                            base = k if desc else 0
                            lo_out, hi_out = (j, 0) if desc else (0, j)
                            nc.vector.tensor_tensor(
                                out=slot_ap(dst, base + lo_out, dims),
                                in0=slot_ap(src, base, dims),
                                in1=slot_ap(src, base + j, dims),
                                op=Alu.min,
                            )
                            nc.vector.tensor_tensor(
                                out=slot_ap(dst, base + hi_out, dims),
                                in0=slot_ap(src, base, dims),
                                in1=slot_ap(src, base + j, dims),
                                op=Alu.max,
                            )
                    src, dst = dst, src

                # halves sorted ascending in `src`: L = slots 0..H-1, R = H..2H-1
                o16 = mpool.tile([128, OUT_C], fp16, tag="o16")
                for ri, r in enumerate(rs):
                    m = mpool.tile([128, K // 2 + 1, OUT_C], fp16, tag=f"m{ri}")
                    sv = src[:]
                    if r > 0:
                        # cands[t] = max(L[t], R[r-1-t]), t = 0..r-1
                        nc.vector.tensor_tensor(
                            out=slot_ap(m, 0, [(1, r)]),
                            in0=slot_ap(src, 0, [(1, r)]),
                            in1=AP(sv.tensor, sv.offset + (H + r - 1) * S,
                                   [sv.ap[0], [-S, r], [1, OUT_C]]),
                            op=Alu.max,
                        )
                    # cands[r] = L[r], cands[r+1] = R[r]
                    nc.vector.tensor_copy(
                        slot_ap(m, r, [(1, 2)]),
                        AP(sv.tensor, sv.offset + r * S,
                           [sv.ap[0], [H * S, 2], [1, OUT_C]]),
                    )
                    # min-reduce cands[0..r+1] into cands[0]
                    n = r + 2
                    while n > 1:
                        a = n - n // 2
                        nc.vector.tensor_tensor(
                            out=slot_ap(m, 0, [(1, n // 2)]),
                            in0=slot_ap(m, 0, [(1, n // 2)]),
                            in1=slot_ap(m, a, [(1, n // 2)]),
                            op=Alu.min,
                        )
                        n = a
                    if ri == 0:
                        nc.vector.tensor_copy(o16[:], slot_ap(m, 0, [(1, 1)]))
                    else:
                        pk = pick_sb[:]
                        pk_ap = AP(pk.tensor, pk.offset + kcol, [pk.ap[0], [0, OUT_C]])
                        kcol += 1
                        nc.vector.copy_predicated(o16[:], pk_ap, slot_ap(m, 0, [(1, 1)]))

                o32 = mpool.tile([128, OUT_C], fp32, tag="o32")
                nc.vector.tensor_copy(o32[:], o16[:])
                nc.sync.dma_start(out[t * 128:(t + 1) * 128, :], o32[:])

    nc.compile()
    return nc


def _prepare(x, kernel, neighbors, degrees):
    """Host-side marshaling: permutation, schedule, idx/mask streams."""
    deg = np.clip(np.asarray(degrees).astype(np.int64), 1, K)
    deg_pad = np.ones(NPAD, np.int64)
    deg_pad[:N] = deg
    # dummies (N..NPAD) have deg 1 but gather only sentinels
    order = np.argsort(-deg_pad, kind="stable")        # global rank -> node id
    deg_sorted = deg_pad[order]
    sched = _make_schedule(deg_sorted)

    # table row of node u: rank j -> core j%8, local slot j//8
    ranks = np.empty(NPAD, np.int64)
    ranks[order] = np.arange(NPAD)
    rho = (ranks % NCORES) * SHARD + ranks // NCORES   # node id -> table row

    nbr = np.asarray(neighbors).astype(np.int64)
    nbr_rows = rho[nbr]                                # [N, K]
    pair_full = np.zeros((NPAD, K), np.int64)
    par_full = np.zeros((NPAD, K), np.int64)
    pair_full[:N] = nbr_rows >> 1
    par_full[:N] = nbr_rows & 1

    xf = np.zeros((NPAD, IN_C), np.float16)
    xf[:N] = np.asarray(x, np.float32).astype(np.float16)
    wf = np.asarray(kernel, np.float32).astype(np.float16)
    infs = np.full((4, OUT_C), np.inf, np.float16)

    karr = np.arange(K, dtype=np.int64)[None, :]

    in_maps = []
    node_of = np.empty((NCORES, SHARD), np.int64)
    for c in range(NCORES):
        nodes_c = order[c::NCORES]                     # local slot i -> node id
        node_of[c] = nodes_c
        d_c = deg_pad[nodes_c]                         # descending
        pair_c = pair_full[nodes_c]                    # [SHARD, K]
        par_c = par_full[nodes_c]
        valid_c = karr < d_c[:, None]                  # [SHARD, K]

        idx_parts = []
        par_parts = []
        pick_parts = []
        for t, (maxd, H, rs) in enumerate(sched):
            sl = slice(t * 128, (t + 1) * 128)
            pt = pair_c[sl, :maxd]                     # [128, maxd]
            vt = valid_c[sl, :maxd]
            stream = np.where(vt, pt, SENT_PAIR).T     # [maxd, 128] slot-major
            wrapped = np.tile(
                stream.reshape(maxd * 8, 16).T, (8, 1)
            )                                          # [128, maxd*8]
            idx_parts.append(wrapped.astype(np.int16))
            par_parts.append(
                np.where(vt, par_c[sl, :maxd], 0).astype(np.int16)
            )                                          # [128, maxd]
            r_t = (d_c[sl] - 1) // 2                   # [128]
            for r in rs[1:]:
                pick_parts.append((r_t == r).astype(np.int16)[:, None])

        idx_all = np.ascontiguousarray(np.concatenate(idx_parts, axis=1))
        par_all = np.ascontiguousarray(np.concatenate(par_parts, axis=1))
        if pick_parts:
            pick_all = np.ascontiguousarray(np.concatenate(pick_parts, axis=1))
        else:
            pick_all = np.zeros((128, 1), np.int16)
        in_maps.append({
            "xT": np.ascontiguousarray(xf[nodes_c].T),
            "w": wf,
            "idx": idx_all,
            "par": par_all,
            "pick": pick_all,
            "infs": infs,
        })

    return sched, in_maps, node_of


def kernel(x, kernel, neighbors, degrees):
    from concourse.bass_utils import run_bass_kernel_spmd

    sched, in_maps, node_of = _prepare(x, kernel, neighbors, degrees)
    if sched not in _CACHE:
        _CACHE[sched] = _emit_program(sched)
    nc = _CACHE[sched]

    res = run_bass_kernel_spmd(nc, in_maps, list(range(NCORES)))
    full = np.empty((NPAD, OUT_C), np.float32)
    for c in range(NCORES):
        full[node_of[c]] = res.results[c]["out"]
    return np.ascontiguousarray(full[:N])
